# revision 29
# baseline (speedup 1.0000x reference)
"""Trainium2 Bass kernel for nn_ClassifierGCN (GCN conv -> z@z^T -> MLP -> sigmoid).

Contract: kernel(**inputs) takes the FULL unsharded inputs (numpy), distributes
across 8 NeuronCores internally, and returns the FULL output (numpy, f32).

Strategy (8 cores), v2:
  - Host: fold D^-1/2 normalization into x rows (src side) and a dense fp8
    edge-count matrix (exact small ints); fold power-of-2 quantization scales
    into the weights so all fc operands stream as fp8-e4m3:
      x' = e4m3(16 * dinv * x),  Wg' = bf16(Wg/16)
      W1 folded for G-symmetry: block-lower-tri(16) packing of vec(G) cuts the
      fc1 contraction 6400 -> 3840 (off-diag blocks use W1[ij]+W1[ji]); x1024
      W2 x 2048; b1 x 4096; b2 x 16384 (core 0 only).
  - Device, phase A (per core): h = x'@Wg' (bf16), aggT = h^T @ C_slice
    (bf16 x fp8), zT = relu(2*(aggT*dinv_d) + 2*bg)  [z scaled by 2 => G by 4]
  - Phase B: per-graph block-tri G = z z^T -> fp8 -> DRAM, ONE AllGather
    (fp8, 246KB out) -> transpose to K-tiles [128, 64 graphs].
  - Phase C1 (fc1): hidden-slice-stationary DoubleRow fp8 matmuls
    (W1 pairs [128,2,128] stationary, G pairs [128,2,64] moving) -> y1T comes
    out already transposed [hid, graphs]; relu+requant to fp8 via one
    activation (scale 2^-9).
  - Phase C2 (fc2): y1T pairs stationary, W2 pairs moving, DoubleRow, output
    [64 graphs, 1600-col chunks]; chunk-outer/K-inner so each chunk is paced
    by its own W2 stream; partials -> bf16 -> ONE ReduceScatter(sum) ->
    sigmoid(scale 2^-14) -> per-core [8, 6400] f32 output.
  - All weight bytes stream once per core (~17MB fp8); full W1 fits SBUF.
"""

import numpy as np
import ml_dtypes

import bass_rust
import concourse.bass as bass
import concourse.mybir as mybir
import concourse.tile as tile
from concourse.bass_utils import run_bass_kernel_spmd
from concourse.masks import make_identity
from concourse.tile_rust import add_dep_helper

# Problem shapes (hardcoded per contract).
N_NEURONS = 80
TBL = 256
LATENT = 128
N_GRAPHS = 64
N_NODES = 5120
N_CORES = 8
GPC = N_GRAPHS // N_CORES          # graphs per core = 8
DPC = N_NODES // N_CORES           # dst nodes per core = 640
N2 = N_NEURONS * N_NEURONS         # 6400
HID = 2 * N2                       # 12800
HS = HID // N_CORES                # hidden slice per core = 1600

# Block-lower-triangular packing of vec(G) (G symmetric): 16-col blocks,
# block-col-major; block b covers rows 16b..79 x cols 16b..16b+16.
BLK = 16
NBLK = N_NEURONS // BLK            # 5
RB = [N_NEURONS - BLK * b for b in range(NBLK)]      # [80, 64, 48, 32, 16]
BLK_OFF = [0]
for b in range(NBLK - 1):
    BLK_OFF.append(BLK_OFF[-1] + BLK * RB[b])        # [0,1280,2304,3072,3584]
PK = BLK_OFF[-1] + BLK * RB[-1]                      # 3840 packed K
PKT = PK // 128                                      # 30 K-tiles
PKP = PKT // 2                                       # 15 K-pair loads

# Quantization scales (powers of two, folded host-side).
SX = 16.0        # x pre-scale (compensated in Wg)
SZ = 2.0         # z scale -> G carries SZ^2 = 4
SW1 = 1024.0
SY = 8.0         # y1 fp8 scale
SW2 = 2048.0
FC1_PSUM = SZ * SZ * SW1           # 4096
FC2_PSUM = SY * SW2                # 16384

DT = mybir.dt.bfloat16
F8 = mybir.dt.float8e4
F32 = mybir.dt.float32
NP_BF = ml_dtypes.bfloat16
NP_F8 = ml_dtypes.float8_e4m3
DR = mybir.MatmulPerfMode.DoubleRow

K_TILES_NODES = N_NODES // 128     # 40
HT = [128] * 12 + [64]             # fc1 hidden tile sizes (12x128 + 64)
HOFF = [sum(HT[:i]) for i in range(13)]              # col offsets in y1T
N_CHUNKS_512 = [(0, 512), (512, 512), (1024, 512), (1536, 64)]  # within 1600
FC2_NC = 4                         # fc2 column chunks of 1600


def _fix_excess_waits(nc):
    """This container's walrus rejects >1 sem-wait on CTRL-class instructions.
    Tile's end-of-context Drain can carry several; move the excess onto NoOp
    carriers inserted just before, same engine, program order preserved."""
    n_fix = 0
    for f in nc.m.functions:
        for bb in f.blocks:
            out, changed = [], False
            for inst in bb.instructions:
                si = inst.sync_info
                waits = list(si.on_wait) if si is not None and si.on_wait else []
                if len(waits) > 1:
                    for w in waits[:-1]:
                        nop = mybir.InstNoOp(name=f"I-waitfix-{n_fix}", ins=[], outs=[])
                        n_fix += 1
                        nop.engine = inst.engine
                        nop.sync_info = bass_rust.SyncInfo(on_wait=[w], on_update=[])
                        out.append(nop)
                    si.on_wait = waits[-1:]
                    changed = True
                out.append(inst)
            if changed:
                bb.instructions = out
    return n_fix


def build_nc(debug=False):
    nc = bass.Bass(num_devices=N_CORES)

    xt8 = nc.dram_tensor("xt8", [TBL, N_NODES], F8, kind="ExternalInput")
    wg = nc.dram_tensor("wg", [TBL, LATENT], DT, kind="ExternalInput")
    bg2 = nc.dram_tensor("bg2", [LATENT, 1], F32, kind="ExternalInput")
    ats = nc.dram_tensor("ats", [N_NODES, DPC], F8, kind="ExternalInput")
    dinv_d = nc.dram_tensor("dinv_d", [128, DPC], F32, kind="ExternalInput")
    w1s = nc.dram_tensor("w1s", [PK, HS], F8, kind="ExternalInput")
    b1s = nc.dram_tensor("b1s", [1, HS], DT, kind="ExternalInput")
    w2s = nc.dram_tensor("w2s", [HS, N2], F8, kind="ExternalInput")
    b2s = nc.dram_tensor("b2s", [1, N2], DT, kind="ExternalInput")
    y = nc.dram_tensor("y", [GPC, N2], F32, kind="ExternalOutput")
    if debug:
        dbg_z = nc.dram_tensor("dbg_z", [128, DPC], F32, kind="ExternalOutput")
        dbg_g = nc.dram_tensor("dbg_g", [128, PKT * 64], F32,
                               kind="ExternalOutput")
        dbg_y1 = nc.dram_tensor("dbg_y1", [128, 13 * 64], F32,
                                kind="ExternalOutput")
        dbg_y1ps = nc.dram_tensor("dbg_y1ps", [128, 13 * 64], F32,
                                  kind="ExternalOutput")
        dbg_w1 = nc.dram_tensor("dbg_w1", [128, 2 * HS], F32,
                                kind="ExternalOutput")

    RG = [list(range(N_CORES))]

    with tile.TileContext(nc) as tc:
        with (
            tc.tile_pool(name="wp1", bufs=1) as wp1,
            tc.tile_pool(name="wp2", bufs=1) as wp2,
            tc.tile_pool(name="const", bufs=1) as constp,
            tc.tile_pool(name="persist", bufs=1) as persist,
            tc.tile_pool(name="dram", bufs=1, space="DRAM") as dramp,
        ):
            # Constants.
            ident = constp.tile([64, 64], F8)
            make_identity(nc, ident[:])
            ones1 = constp.tile([1, 64], DT)
            nc.gpsimd.memset(ones1[:], 1.0)
            bg2_sb = constp.tile([LATENT, 1], F32)
            dinv_d_sb = constp.tile([128, DPC], F32)
            b1_sb = constp.tile([1, HS], DT)
            b2_sb = constp.tile([1, N2], DT)

            # Persistent SBUF tensors.
            zT = persist.tile([128, DPC], DT)            # [latent, local node]
            gT_big = persist.tile([128, PKT * 64], F8)   # packed-G K-tiles x 64
            y1T = persist.tile([128, 13 * 64], F8)       # y1^T tiles (fp8, xSY)

            # DRAM bounce buffers for the collectives.
            g_loc = dramp.tile([GPC, PK], F8, name="g_loc")
            g_all = dramp.tile([N_GRAPHS, PK], F8, addr_space="Shared",
                               name="g_all")
            y_loc = dramp.tile([N_GRAPHS, N2], DT, name="y_loc")
            y_red = dramp.tile([GPC, N2], DT, name="y_red")

            # ---- Phase A: h = x'@Wg', aggT = h^T @ C, z = relu(...) ----
            with (
                tc.tile_pool(name="xa", bufs=3) as xap,
                tc.tile_pool(name="hp", bufs=3) as hpool,
                tc.tile_pool(name="atp", bufs=4) as atp,
                tc.tile_pool(name="hps", bufs=2, space="PSUM") as hps,
                tc.tile_pool(name="aggps", bufs=1, space="PSUM") as aggps,
            ):
                wg_sb = xap.tile([128, 2, LATENT], DT, tag="wg", bufs=1)
                nc.sync.dma_start(
                    wg_sb[:], wg[:, :].rearrange("(a b) c -> b a c", a=2))
                agg = aggps.tile([128, DPC], F32)
                xt_c = None
                at_blocks = {}
                XT_CHUNKS = [(0, 2), (2, 8), (10, 10), (20, 10), (30, 10)]
                xt_starts = {c0: n for c0, n in XT_CHUNKS}
                k0 = 0
                ph4 = None
                pending_agg = None

                def _emit_agg(h_sb, kbase):
                    for j in range(2):
                        p = kbase // 2 + j
                        st = (p == 0)
                        sp = (p == K_TILES_NODES // 2 - 1)
                        hpair = h_sb[:, j * 256:(j + 1) * 256].rearrange(
                            "q (two f) -> q two f", two=2)
                        atb = at_blocks[p]
                        nc.tensor.matmul(agg[:, 0:512], lhsT=hpair,
                                         rhs=atb[:, :, 0:512],
                                         start=st, stop=sp,
                                         skip_group_check=not st,
                                         perf_mode=DR)
                        nc.tensor.matmul(agg[:, 512:640], lhsT=hpair,
                                         rhs=atb[:, :, 512:640],
                                         start=st, stop=sp,
                                         skip_group_check=not st,
                                         perf_mode=DR)

                for k in range(K_TILES_NODES):
                    if k in xt_starts:
                        n = xt_starts[k]
                        cs, ce = k * 128, (k + n) * 128
                        xt_c = xap.tile([128, 2, 1280], F8, tag="xt")
                        k0 = k
                        nc.sync.dma_start(
                            xt_c[:, :, 0:n * 128],
                            xt8[:, cs:ce].rearrange("(a b) c -> b a c", a=2))
                    if k % 2 == 0:
                        kb = k // 2
                        at2 = atp.tile([128, 2, DPC], F8)
                        at_blocks[kb] = at2
                        src = ats[k * 128:(k + 2) * 128, :].rearrange(
                            "(a b) c -> b a c", a=2)
                        ats_dma = nc.sync.dma_start(at2[:], src)
                        if kb == K_TILES_NODES // 2 - 1:
                            nc._ats_last_dma = ats_dma.ins
                    if k == 5:
                        nc.sync.dma_start(bg2_sb[:], bg2[:, :])
                        nc.sync.dma_start(dinv_d_sb[:], dinv_d[:, :])
                    mm = (k - k0) * 128
                    if k % 4 == 0:
                        ph4 = hps.tile([128, 512], F32)
                    sl = ph4[:, (k % 4) * 128:(k % 4 + 1) * 128]
                    mm0 = nc.tensor.matmul(sl, lhsT=xt_c[:, 0, mm:mm + 128],
                                           rhs=wg_sb[:, 0, :],
                                           start=(k % 4 == 0), stop=False,
                                           skip_group_check=(k % 4 != 0))
                    if k % 4 == 0:
                        h_start_mm = mm0
                    else:
                        # start=True clears the WHOLE psum bank's has_written;
                        # order every sub-group behind the bank opener.
                        add_dep_helper(mm0.ins, h_start_mm.ins, sync=True,
                                       reason="h-psum bank opener first")
                    nc.tensor.matmul(sl, lhsT=xt_c[:, 1, mm:mm + 128],
                                     rhs=wg_sb[:, 1, :],
                                     start=False, stop=(k % 4 == 3),
                                     skip_group_check=True)
                    if k % 4 == 3:
                        h_sb = hpool.tile([128, 512], F8)
                        nc.vector.tensor_copy(h_sb[:], ph4[:])
                        # software pipeline: agg mms for the PREVIOUS group are
                        # emitted here so PE has back-to-back work while this
                        # group's DVE copy completes (keeps the p-state warm)
                        if pending_agg is not None:
                            _emit_agg(pending_agg[0], pending_agg[1])
                        pending_agg = (h_sb, k - 3)
                if pending_agg is not None:
                    _emit_agg(pending_agg[0], pending_agg[1])
                nc.sync.dma_start(b1_sb[:], b1s[:, :])
                nc.sync.dma_start(b2_sb[:], b2s[:, :])
                aggs = xap.tile([128, DPC], F32, tag="aggs", bufs=1)
                nc.vector.tensor_tensor(aggs[:], agg[:], dinv_d_sb[:],
                                        op=mybir.AluOpType.mult)
                nc.scalar.activation(zT[:], aggs[:],
                                     mybir.ActivationFunctionType.Relu,
                                     bias=bg2_sb[:, 0:1], scale=SZ / SX)

            # ---- Phase B: block-tri G = z z^T -> fp8 -> DRAM -> AllGather ----
            with (
                tc.tile_pool(name="gps", bufs=NBLK, space="PSUM") as gps,
                tc.tile_pool(name="gsb", bufs=NBLK) as gsbp,
            ):
                for b in range(NBLK):
                    gp = gps.tile([RB[b], GPC * BLK], F32)
                    g_start_mm = None
                    for g in range(GPC):
                        zg0 = g * N_NEURONS
                        lhs = zT[:, zg0 + BLK * b:zg0 + N_NEURONS]
                        rhs = zT[:, zg0 + BLK * b:zg0 + BLK * (b + 1)]
                        gmm = nc.tensor.matmul(gp[:, g * BLK:(g + 1) * BLK],
                                               lhsT=lhs, rhs=rhs,
                                               start=(g == 0), stop=(g == GPC - 1),
                                               skip_group_check=(g != 0))
                        if g == 0:
                            g_start_mm = gmm
                        else:
                            add_dep_helper(gmm.ins, g_start_mm.ins, sync=True,
                                           reason="G-psum bank opener first")
                    gsb = gsbp.tile([RB[b], GPC * BLK], F8, name=f"gsb{b}",
                                    bufs=1)
                    nc.vector.tensor_copy(gsb[:], gp[:])
                    # pack to g_loc[g, BLK_OFF[b] + i*16 + jj]
                    nc.sync.dma_start(
                        g_loc[:, BLK_OFF[b]:BLK_OFF[b] + RB[b] * BLK]
                        .rearrange("g (i j) -> i g j", j=BLK),
                        gsb[:].rearrange("i (g j) -> i g j", g=GPC))
                nc.gpsimd.collective_compute(
                    "AllGather", mybir.AluOpType.bypass, replica_groups=RG,
                    ins=[g_loc.opt()], outs=[g_all.opt()],
                )

            # W2 stream: issued on the sync queue right here (program order:
            # after the g_loc stores, before anything gated on collectives) so
            # its transfers fill the DMA engine as soon as W1 finishes.
            w2_tiles = []
            w2_tails = []
            for n in range(FC2_NC):
                c0 = n * 1600
                for u in range(6):
                    w2t = wp2.tile([128, 2, 1600], F8, tag="w2", bufs=24)
                    nc.sync.dma_start(
                        w2t[:],
                        w2s[u * 256:(u + 1) * 256,
                            c0:c0 + 1600].rearrange("(a b) c -> b a c", a=2))
                    w2_tiles.append(w2t)
                w2tt = wp2.tile([64, 1600], F8, tag="w2t", bufs=4)
                nc.sync.dma_start(w2tt[:], w2s[1536:1600, c0:c0 + 1600])
                w2_tails.append(w2tt)

            # ---- Phase C0: transpose Gall into [128 x 64] K-tiles ----
            with (
                tc.tile_pool(name="gallp", bufs=2) as gallp,
                tc.tile_pool(name="tps", bufs=4, space="PSUM") as tps,
            ):
                for c in range(2):
                    ga = gallp.tile([N_GRAPHS, PK // 2], F8)
                    nc.gpsimd.dma_start(ga[:], g_all[:, c * (PK // 2):(c + 1) * (PK // 2)])
                    for j in range(PKT // 2):
                        t = c * (PKT // 2) + j
                        tp = tps.tile([128, N_GRAPHS, 2], F8)
                        nc.tensor.transpose(tp[:, :, 0:1],
                                            ga[:, j * 128:(j + 1) * 128],
                                            ident[:])
                        nc.vector.tensor_copy(gT_big[:, t * 64:(t + 1) * 64],
                                              tp[:, :, 0])

                # ---- Phase C1 (fc1): DoubleRow, W1 stationary, out y1T ----
                with tc.tile_pool(name="y1ps", bufs=1, space="PSUM") as y1psp:
                    y1ps = y1psp.tile([128, 13 * 64], F32)  # [128, 832]
                    bank_start = {}
                    for h in range(13):
                        # b1 into PSUM: out[hid,g] += b1[hid] * ones[g]
                        st = h in (0, 8)
                        omm = nc.tensor.matmul(
                            y1ps[0:HT[h], h * 64:(h + 1) * 64],
                            lhsT=b1_sb[0:1, h * 128:h * 128 + HT[h]],
                            rhs=ones1[:], start=st, stop=False,
                            skip_group_check=not st)
                        bank = 0 if h < 8 else 1
                        if st:
                            bank_start[bank] = omm
                        else:
                            add_dep_helper(omm.ins, bank_start[bank].ins,
                                           sync=True,
                                           reason="fc1 psum bank opener first")
                    for t in range(PKP):
                        w1t = wp1.tile([128, 2, HS], F8, tag="w1", bufs=PKP)
                        w1_dma = nc.scalar.dma_start(
                            w1t[:],
                            w1s[t * 256:(t + 1) * 256, :].rearrange(
                                "(a b) c -> b a c", a=2))
                        if t == 0:
                            add_dep_helper(w1_dma.ins, nc._ats_last_dma,
                                           sync=True,
                                           reason="w1 stream after phase-A loads")
                            nc._dbg_w1t0 = w1t
                        if t == PKP - 1:
                            nc._w1_last_dma = w1_dma.ins
                        gpair = gT_big[:, (2 * t) * 64:(2 * t + 2) * 64]\
                            .rearrange("p (two f) -> p two f", two=2)
                        for h in range(13):
                            nc.tensor.matmul(
                                y1ps[0:HT[h], h * 64:(h + 1) * 64],
                                lhsT=w1t[:, :, h * 128:h * 128 + HT[h]],
                                rhs=gpair, start=False, stop=(t == PKP - 1),
                                skip_group_check=True, perf_mode=DR)
                    if debug:
                        with tc.tile_pool(name="dbgps", bufs=1) as dbgpsp:
                            dps = dbgpsp.tile([128, 13 * 64], F32)
                            nc.vector.tensor_copy(dps[:], y1ps[:])
                            nc.sync.dma_start(dbg_y1ps[:, :], dps[:])
                            dw1 = dbgpsp.tile([128, 2 * HS], F32)
                            nc.vector.tensor_copy(
                                dw1[:], nc._dbg_w1t0[:].rearrange(
                                    "p a c -> p (a c)"))
                            nc.sync.dma_start(dbg_w1[:, :], dw1[:])
                    # relu + requant to fp8 (scale SY / FC1_PSUM = 2^-9)
                    nc.scalar.activation(y1T[:, 0:768], y1ps[:, 0:768],
                                         mybir.ActivationFunctionType.Relu,
                                         scale=SY / FC1_PSUM)
                    nc.scalar.activation(y1T[0:64, 768:832], y1ps[0:64, 768:832],
                                         mybir.ActivationFunctionType.Relu,
                                         scale=SY / FC1_PSUM)

            # ---- Phase C2 (fc2): chunk-outer, K-inner, DoubleRow ----
            with (
                tc.tile_pool(name="p2ps", bufs=2, space="PSUM") as p2psp,
                tc.tile_pool(name="y2sb", bufs=2) as y2sbp,
                tc.tile_pool(name="sig", bufs=1) as sigp,
            ):
                for n in range(FC2_NC):
                    c0 = n * 1600
                    p2 = p2psp.tile([N_GRAPHS, 1600], F32)
                    for (n0, nw) in N_CHUNKS_512:
                        nc.tensor.matmul(p2[:, n0:n0 + nw], lhsT=ones1[:],
                                         rhs=b2_sb[:, c0 + n0:c0 + n0 + nw],
                                         start=True, stop=False)
                    for u in range(6):
                        w2t = w2_tiles[n * 6 + u]
                        y1pair = y1T[:, (2 * u) * 64:(2 * u + 2) * 64]\
                            .rearrange("p (two f) -> p two f", two=2)
                        for (n0, nw) in N_CHUNKS_512:
                            nc.tensor.matmul(
                                p2[:, n0:n0 + nw], lhsT=y1pair,
                                rhs=w2t[:, :, n0:n0 + nw],
                                start=False, stop=False,
                                skip_group_check=True, perf_mode=DR)
                    w2tt = w2_tails[n]
                    for (n0, nw) in N_CHUNKS_512:
                        nc.tensor.matmul(
                            p2[:, n0:n0 + nw],
                            lhsT=y1T[0:64, 768:832],
                            rhs=w2tt[0:64, n0:n0 + nw],
                            start=False, stop=True,
                            skip_group_check=True)
                    y2sb = y2sbp.tile([N_GRAPHS, 1600], DT)
                    nc.vector.tensor_copy(y2sb[:], p2[:])
                    nc.sync.dma_start(y_loc[:, c0:c0 + 1600], y2sb[:])

                nc.gpsimd.collective_compute(
                    "ReduceScatter", mybir.AluOpType.add, replica_groups=RG,
                    ins=[y_loc.opt()], outs=[y_red.opt()],
                )
                # sigmoid over all 128 partitions: [8, 6400] -> [128, 400]
                ys = sigp.tile([128, 400], DT)
                nc.sync.dma_start(
                    ys[:], y_red[:, :].rearrange("g (j t) -> g j t", j=16))
                yo = sigp.tile([128, 400], F32)
                nc.scalar.activation(yo[:], ys[:],
                                     mybir.ActivationFunctionType.Sigmoid,
                                     scale=1.0 / FC2_PSUM)
                nc.sync.dma_start(
                    y[:, :].rearrange("g (j t) -> g j t", j=16), yo[:])

            if debug:
                with tc.tile_pool(name="dbgp", bufs=1) as dbgp:
                    dz = dbgp.tile([128, DPC], F32)
                    nc.vector.tensor_copy(dz[:], zT[:])
                    nc.sync.dma_start(dbg_z[:, :], dz[:])
                    dg = dbgp.tile([128, PKT * 64], F32)
                    nc.vector.tensor_copy(dg[:], gT_big[:])
                    nc.sync.dma_start(dbg_g[:, :], dg[:])
                    dy1 = dbgp.tile([128, 13 * 64], F32)
                    nc.vector.tensor_copy(dy1[:], y1T[:])
                    nc.sync.dma_start(dbg_y1[:, :], dy1[:])

    _fix_excess_waits(nc)
    return nc


_NC_CACHE = None


def _get_nc():
    global _NC_CACHE
    if _NC_CACHE is None:
        _NC_CACHE = build_nc()
    return _NC_CACHE


def _q8(a, scale):
    return np.clip(np.asarray(a, np.float32) * scale, -240.0, 240.0).astype(NP_F8)


def prep_in_maps(x, edge_index, Wg, bg, W1, b1, W2, b2):
    x = np.asarray(x, np.float32)
    edge_index = np.asarray(edge_index)
    Wg = np.asarray(Wg, np.float32)
    bg = np.asarray(bg, np.float32)
    W1 = np.asarray(W1, np.float32)
    b1 = np.asarray(b1, np.float32)
    W2 = np.asarray(W2, np.float32)
    b2 = np.asarray(b2, np.float32)

    src = edge_index[0].astype(np.int64)
    dst = edge_index[1].astype(np.int64)

    deg = np.bincount(dst, minlength=N_NODES).astype(np.float32)
    dinv = np.where(deg > 0, 1.0 / np.sqrt(np.maximum(deg, 1.0)), 0.0).astype(np.float32)

    # Dense edge-count matrix [src, dst]; small ints exact in fp8.
    counts = np.bincount(src * N_NODES + dst, minlength=N_NODES * N_NODES)
    at = counts.astype(NP_F8).reshape(N_NODES, N_NODES)

    # x' = dinv-scaled x, fp8 (x SX); the 1/SX compensation is folded into
    # the z activation scale (h streams as fp8 at the x-SX scale).
    xt8 = np.ascontiguousarray(_q8((dinv[:, None] * x).T, SX))
    wg_np = Wg.astype(NP_BF)
    bg2_np = np.ascontiguousarray((SZ * bg).reshape(LATENT, 1))

    # W1 block-tri fold: packed index p -> (i, j); off-diag blocks folded.
    ii = np.concatenate([np.repeat(np.arange(BLK * b, N_NEURONS), BLK)
                         for b in range(NBLK)])
    jj = np.concatenate([np.tile(np.arange(BLK * b, BLK * (b + 1)), RB[b])
                         for b in range(NBLK)])
    W1f = W1[ii * N_NEURONS + jj].copy()
    off = (ii // BLK) != (jj // BLK)
    W1f[off] += W1[jj[off] * N_NEURONS + ii[off]]
    W1f8 = _q8(W1f, SW1)
    W28 = _q8(W2, SW2)

    in_maps = []
    for c in range(N_CORES):
        s0 = c * HS
        b2c = b2 if c == 0 else np.zeros_like(b2)
        in_maps.append({
            "xt8": xt8,
            "wg": wg_np,
            "bg2": bg2_np,
            "ats": np.ascontiguousarray(at[:, c * DPC:(c + 1) * DPC]),
            "dinv_d": np.ascontiguousarray(np.broadcast_to(
                dinv[c * DPC:(c + 1) * DPC], (128, DPC))),
            "w1s": np.ascontiguousarray(W1f8[:, s0:s0 + HS]),
            "b1s": np.ascontiguousarray(
                (b1[s0:s0 + HS] * FC1_PSUM).reshape(1, HS)).astype(NP_BF),
            "w2s": np.ascontiguousarray(W28[s0:s0 + HS, :]),
            "b2s": np.ascontiguousarray(
                (b2c * FC2_PSUM).reshape(1, N2)).astype(NP_BF),
        })
    return in_maps


def kernel(x, edge_index, Wg, bg, W1, b1, W2, b2):
    in_maps = prep_in_maps(x, edge_index, Wg, bg, W1, b1, W2, b2)
    nc = _get_nc()
    res = run_bass_kernel_spmd(nc, in_maps, core_ids=list(range(N_CORES)))
    out = np.concatenate([res.results[c]["y"] for c in range(N_CORES)], axis=0)
    return out.reshape(-1).astype(np.float32)
